# revision 3
# baseline (speedup 1.0000x reference)
"""Trainium2 Bass kernel for DiffeomorphicLearner (gnn_message_passing).

Math (per step t, T=8 steps):
    sq_i  = ||z_i||^2
    K_ij  = exp((2 z_i.z_j - sq_i - sq_j) / (2 rho^2))
    v     = Z @ Aaff_t.T + b_t + K @ A_t
    Z    <- Z + DT * v

Distribution: row-parallel over N=4096 across 8 cores (512 rows each).
Each core keeps its Z rows (fp32 master, stored TRANSPOSED as [D, n_loc])
and computes K^T slices [j, i_loc] against an all-gathered bf16 copy of
the full Z^T plus column-layout sq. One bf16 AllGather of [Z^T; sq]
(257 x 512 per rank) per step.

Precision: all matmuls bf16 in / fp32 PSUM accumulate; exp argument and
state updates fp32. sq_j enters as a per-partition ACT bias; the
exp(-c*sq_i) factor is factored out per-column and applied after the
K@A contraction (it is constant along j).
"""

import numpy as np
import ml_dtypes

import concourse.bass as bass
import concourse.tile as tile
from concourse import bacc, mybir
from concourse import bass_utils

BF16NP = ml_dtypes.bfloat16

N_CORES = 8
N, D, T = 4096, 256, 8
RHO = 16.0
DT = 1.0 / T
CEXP = 1.0 / (2.0 * RHO * RHO)  # 1/512

NLOC = N // N_CORES            # 512 rows per core
NJB = N // 128                 # 32 j-blocks of 128
NJB_LOC = NLOC // 128          # 4 local i-blocks
PAY_R = D + 1                  # payload rows: 256 Z rows + 1 sq row

F32 = mybir.dt.float32
BF16 = mybir.dt.bfloat16

_CACHED = {}


def _build():
    """Build the 8-core SPMD Bass program (same program on every core)."""
    nc = bacc.Bacc("TRN2", target_bir_lowering=False, debug=False,
                   num_devices=N_CORES)

    # ---- DRAM I/O -------------------------------------------------------
    zt_local0 = nc.dram_tensor("zt_local0", [D, NLOC], F32, kind="ExternalInput")
    zb_local0 = nc.dram_tensor("zb_local0", [D, NLOC], BF16, kind="ExternalInput")
    ztb_full0 = nc.dram_tensor("ztb_full0", [D, N], BF16, kind="ExternalInput")
    bias_col0 = nc.dram_tensor("bias_col0", [128, NJB], F32, kind="ExternalInput")
    e_row0 = nc.dram_tensor("e_row0", [1, NLOC], BF16, kind="ExternalInput")
    a_b = nc.dram_tensor("a_b", [T, N, D], BF16, kind="ExternalInput")
    aaff_b = nc.dram_tensor("aaff_b", [T, D, D], BF16, kind="ExternalInput")
    b_b = nc.dram_tensor("b_b", [T, 1, D], BF16, kind="ExternalInput")
    ones_col = nc.dram_tensor("ones_col", [128, 1], BF16, kind="ExternalInput")
    ones_row = nc.dram_tensor("ones_row", [1, NLOC], BF16, kind="ExternalInput")
    out_zt = nc.dram_tensor("out_zt", [D, NLOC], F32, kind="ExternalOutput")

    EXP = mybir.ActivationFunctionType.Exp

    with tile.TileContext(nc) as tc:
        with tc.tile_pool(name="persist", bufs=1) as persist, \
             tc.tile_pool(name="state", bufs=2) as state, \
             tc.tile_pool(name="astream", bufs=64) as astream, \
             tc.tile_pool(name="kpool", bufs=6) as kpool, \
             tc.tile_pool(name="work", bufs=2) as work, \
             tc.tile_pool(name="psum", bufs=1, space="PSUM") as psum, \
             tc.tile_pool(name="dram", bufs=2, space="DRAM") as dram:

            # ---- constants / persistent buffers -------------------------
            onec = persist.tile([128, 1], BF16, name="onec")
            nc.sync.dma_start(onec[:], ones_col[:])
            oner = persist.tile([1, NLOC], BF16, name="oner")
            nc.sync.dma_start(oner[:], ones_row[:])

            # full gathered Z^T (bf16), rewritten each step by unpack DMAs
            ztf = [persist.tile([128, N], BF16, name=f"ztf{ch}") for ch in (0, 1)]
            for ch in (0, 1):
                nc.sync.dma_start(ztf[ch][:], ztb_full0[ch * 128:(ch + 1) * 128, :])

            # gathered sq in column layout -> prescaled ACT bias (-c * sq)
            bias_col = persist.tile([128, NJB], F32, name="bias_col")
            nc.sync.dma_start(bias_col[:], bias_col0[:])
            sqc_all = persist.tile([128, NJB], BF16, name="sqc_all")

            # local state: fp32 master + bf16 working copy
            zt = [state.tile([128, NLOC], F32, name=f"zt{ch}", tag=f"zt{ch}")
                  for ch in (0, 1)]
            for ch in (0, 1):
                nc.sync.dma_start(zt[ch][:], zt_local0[ch * 128:(ch + 1) * 128, :])
            zb = [state.tile([128, NLOC], BF16, name=f"zb{ch}", tag=f"zb{ch}")
                  for ch in (0, 1)]
            for ch in (0, 1):
                nc.sync.dma_start(zb[ch][:], zb_local0[ch * 128:(ch + 1) * 128, :])

            e_row = state.tile([1, NLOC], BF16, name="e_row", tag="e_row")
            nc.sync.dma_start(e_row[:], e_row0[:])

            for t in range(T):
                last = (t == T - 1)

                # ---- E broadcast: E[p, i] = exp(-c*sq_i) ----------------
                e_ps = psum.tile([128, NLOC], F32, name=f"e_ps_{t}", tag="e",
                                 bufs=1)
                nc.tensor.matmul(e_ps[:], oner[:, 0:128], e_row[:],
                                 start=True, stop=True)
                e_sb = work.tile([128, NLOC], F32, name=f"e_sb_{t}", tag="e_sb",
                                 bufs=2)
                nc.scalar.activation(e_sb[:], e_ps[:],
                                     mybir.ActivationFunctionType.Copy)

                # ---- affine part: va[dh] = Aaff_t @ z_loc + b_t ---------
                aaff_t = [astream.tile([128, D], BF16, name=f"aaff_{t}_{ch}",
                                       tag="aaff", bufs=4) for ch in (0, 1)]
                for ch in (0, 1):
                    nc.sync.dma_start(
                        aaff_t[ch][:], aaff_b.ap()[t, ch * 128:(ch + 1) * 128, :])
                brow_t = astream.tile([1, D], BF16, name=f"brow_{t}", tag="brow",
                                      bufs=2)
                nc.sync.dma_start(brow_t[:], b_b.ap()[t, :, :])

                va = [psum.tile([128, NLOC], F32, name=f"va_{t}_{dh}",
                                tag=f"va{dh}", bufs=1) for dh in (0, 1)]
                for dh in (0, 1):
                    nc.tensor.matmul(va[dh][:],
                                     aaff_t[0][:, dh * 128:(dh + 1) * 128],
                                     zb[0][:], start=True, stop=False)
                    nc.tensor.matmul(va[dh][:],
                                     aaff_t[1][:, dh * 128:(dh + 1) * 128],
                                     zb[1][:], start=False, stop=False)
                    nc.tensor.matmul(va[dh][:],
                                     brow_t[:, dh * 128:(dh + 1) * 128],
                                     oner[:], start=False, stop=True)

                # ---- main loop over j-blocks ----------------------------
                vr = [psum.tile([128, NLOC], F32, name=f"vr_{t}_{dh}",
                                tag=f"vr{dh}", bufs=1) for dh in (0, 1)]
                for jb in range(NJB):
                    s_ps = psum.tile([128, NLOC], F32, name=f"s_{t}_{jb}",
                                     tag="s", bufs=2)
                    nc.tensor.matmul(s_ps[:],
                                     ztf[0][:, jb * 128:(jb + 1) * 128],
                                     zb[0][:], start=True, stop=False)
                    nc.tensor.matmul(s_ps[:],
                                     ztf[1][:, jb * 128:(jb + 1) * 128],
                                     zb[1][:], start=False, stop=True)
                    # K^T[j, i] (without the exp(-c*sq_i) factor)
                    k_t = kpool.tile([128, NLOC], BF16, name=f"k_{t}_{jb}",
                                     tag="k")
                    nc.scalar.activation(k_t[:], s_ps[:], EXP,
                                         scale=2.0 * CEXP,
                                         bias=bias_col[:, jb:jb + 1])
                    a_t = astream.tile([128, D], BF16, name=f"a_{t}_{jb}",
                                       tag="a")
                    nc.sync.dma_start(
                        a_t[:], a_b.ap()[t, jb * 128:(jb + 1) * 128, :])
                    for dh in (0, 1):
                        nc.tensor.matmul(vr[dh][:],
                                         a_t[:, dh * 128:(dh + 1) * 128],
                                         k_t[:],
                                         start=(jb == 0),
                                         stop=(jb == NJB - 1 and dh == 1))

                # ---- update: z <- z + va + vr * E -----------------------
                zt_new = [state.tile([128, NLOC], F32, name=f"ztn_{t}_{ch}",
                                     tag=f"zt{ch}") for ch in (0, 1)]
                for dh in (0, 1):
                    t1 = work.tile([128, NLOC], F32, name=f"t1_{t}_{dh}",
                                   tag="t1", bufs=2)
                    nc.vector.tensor_mul(t1[:], vr[dh][:], e_sb[:])
                    t2 = work.tile([128, NLOC], F32, name=f"t2_{t}_{dh}",
                                   tag="t2", bufs=2)
                    nc.vector.tensor_add(t2[:], va[dh][:], zt[dh][:])
                    nc.vector.tensor_add(zt_new[dh][:], t1[:], t2[:])
                zt = zt_new

                if last:
                    for ch in (0, 1):
                        nc.sync.dma_start(
                            out_zt[ch * 128:(ch + 1) * 128, :], zt[ch][:])
                    break

                # ---- post-update tail: bf16 copy, sq, payload, AG -------
                zb_new = [state.tile([128, NLOC], BF16, name=f"zbn_{t}_{ch}",
                                     tag=f"zb{ch}") for ch in (0, 1)]
                z2 = [work.tile([128, NLOC], BF16, name=f"z2_{t}_{ch}",
                                tag=f"z2{ch}", bufs=2) for ch in (0, 1)]
                for ch in (0, 1):
                    nc.vector.tensor_copy(zb_new[ch][:], zt[ch][:])
                    nc.vector.tensor_mul(z2[ch][:], zb_new[ch][:], zb_new[ch][:])
                zb = zb_new

                # sq in column layout [128, 4] (for payload -> bias)
                sqc_ps = psum.tile([128, NJB_LOC], F32, name=f"sqc_{t}",
                                   tag="sq", bufs=1)
                for ib in range(NJB_LOC):
                    for ch in (0, 1):
                        nc.tensor.matmul(sqc_ps[:, ib:ib + 1],
                                         z2[ch][:, ib * 128:(ib + 1) * 128],
                                         onec[:],
                                         start=(ch == 0), stop=(ch == 1))
                sqc_b = work.tile([128, NJB_LOC], BF16, name=f"sqcb_{t}",
                                  tag="sqcb", bufs=2)
                nc.scalar.activation(sqc_b[:], sqc_ps[:],
                                     mybir.ActivationFunctionType.Copy)

                # sq in row layout [1, 512] -> E row for next step
                sqr_ps = psum.tile([1, NLOC], F32, name=f"sqr_{t}",
                                   tag="sq", bufs=1)
                for ch in (0, 1):
                    nc.tensor.matmul(sqr_ps[:], onec[:], z2[ch][:],
                                     start=(ch == 0), stop=(ch == 1))
                e_row_new = state.tile([1, NLOC], BF16, name=f"er_{t}",
                                       tag="e_row")
                nc.scalar.activation(e_row_new[:], sqr_ps[:], EXP, scale=-CEXP)
                e_row = e_row_new

                # ---- pack payload + AllGather ---------------------------
                cc_in = dram.tile([PAY_R, NLOC], BF16, name=f"cci_{t}",
                                  tag="cci")
                cc_out = dram.tile([N_CORES * PAY_R, NLOC], BF16,
                                   name=f"cco_{t}", tag="cco",
                                   addr_space="Shared")
                for ch in (0, 1):
                    nc.sync.dma_start(cc_in[ch * 128:(ch + 1) * 128, :],
                                      zb[ch][:])
                nc.sync.dma_start(
                    cc_in[D:D + 1, :].rearrange("o (p b) -> o p b", b=NJB_LOC),
                    sqc_b[:])
                nc.gpsimd.collective_compute(
                    "AllGather", mybir.AluOpType.bypass,
                    replica_groups=[list(range(N_CORES))],
                    ins=[cc_in[:].opt()], outs=[cc_out[:].opt()],
                )

                # ---- unpack gathered Z^T, sq ----------------------------
                for r in range(N_CORES):
                    base = r * PAY_R
                    for ch in (0, 1):
                        nc.sync.dma_start(
                            ztf[ch][:, r * NLOC:(r + 1) * NLOC],
                            cc_out[base + ch * 128:base + (ch + 1) * 128, :])
                    nc.sync.dma_start(
                        sqc_all[:, r * NJB_LOC:(r + 1) * NJB_LOC],
                        cc_out[base + D:base + D + 1, :].rearrange(
                            "o (p b) -> o p b", b=NJB_LOC))
                # bias = -c * sq  (fp32, per-partition columns)
                nc.vector.tensor_scalar_mul(bias_col[:], sqc_all[:], -CEXP)

    nc.compile()
    return nc


def _prepare_in_maps(X, A, A_aff, b_aff):
    XT = np.ascontiguousarray(X.T.astype(np.float32))          # [D, N]
    sq0 = (X.astype(np.float32) ** 2).sum(axis=1)              # [N]
    ztb_full0 = XT.astype(BF16NP)
    bias_col0 = np.ascontiguousarray(
        (-CEXP * sq0).reshape(NJB, 128).T.astype(np.float32))  # [128, 32]
    a_b = (DT * A.astype(np.float32)).astype(BF16NP)           # [T, N, D]
    aaff_b = np.ascontiguousarray(
        (DT * A_aff.astype(np.float32)).transpose(0, 2, 1)).astype(BF16NP)
    b_b = (DT * b_aff.astype(np.float32)).reshape(T, 1, D).astype(BF16NP)
    ones_col = np.ones((128, 1), dtype=BF16NP)
    ones_row = np.ones((1, NLOC), dtype=BF16NP)

    in_maps = []
    for c in range(N_CORES):
        cols = slice(c * NLOC, (c + 1) * NLOC)
        zt_local0 = np.ascontiguousarray(XT[:, cols])
        in_maps.append({
            "zt_local0": zt_local0,
            "zb_local0": zt_local0.astype(BF16NP),
            "ztb_full0": ztb_full0,
            "bias_col0": bias_col0,
            "e_row0": np.exp(-CEXP * sq0[cols])[None, :].astype(BF16NP),
            "a_b": a_b,
            "aaff_b": aaff_b,
            "b_b": b_b,
            "ones_col": ones_col,
            "ones_row": ones_row,
        })
    return in_maps


def _get_nc():
    if "nc" not in _CACHED:
        _CACHED["nc"] = _build()
    return _CACHED["nc"]


def kernel(X, A, A_aff, b_aff):
    nc = _get_nc()
    in_maps = _prepare_in_maps(X, A, A_aff, b_aff)
    res = bass_utils.run_bass_kernel_spmd(
        nc, in_maps, core_ids=list(range(N_CORES)))
    out = np.empty((N, D), dtype=np.float32)
    for c in range(N_CORES):
        out[c * NLOC:(c + 1) * NLOC, :] = res.results[c]["out_zt"].T
    return out
